# revision 30
# baseline (speedup 1.0000x reference)
"""v5: skewed pair pipeline + pair-couple merged A/D evacs + PE LDW interleave.

Per channel c: Y = D @ f(U @ X @ U.T) @ D.T, bf16, 2 channels per pair,
pairs software-pipelined with 3-stage skew.  Pair-couples (2j, 2j+1) share
a [128,1024] pA tile (one FD-1024 A-evac) and a [128,512] pY tile (one
FD-512 D-evac).  PE emission interleaves the 8 tiny banded C-MMs between
longer A/B/D MMs so LDWEIGHTS prefetch stays ahead.

Host layout [B,H,C,W]; input chunks' DMAs spread over early iterations so
chunk 0 completes first; output DMA per half-chunk (4 pairs).

PSUM banks: pA2 [128,1024] + pZ [128,1024]x2 + pS [128,512] + pY [128,512] = 8.
Evac: DVE {A-merged FD-1024 (odd pairs), C FD-512}; ACT {Lrelu FD-1024,
D-merged FD-512 (every other)}.
"""
import numpy as np
import ml_dtypes
from contextlib import ExitStack

import concourse.bacc as bacc
import concourse.tile as tile
from concourse import mybir
from concourse.bass_utils import run_bass_kernel_spmd

F32 = mybir.dt.float32
BF16 = mybir.dt.bfloat16
AF = mybir.ActivationFunctionType

N_CORES = 8
B_CORE = 2
H = W = C = 128
NEG_SLOPE = 0.01
NCHUNK = 8
CPC = C // NCHUNK


def _keys_cubic(x):
    x = np.abs(x)
    return np.where(
        x <= 1, (1.5 * x - 2.5) * x * x + 1,
        np.where(x < 2, ((-0.5 * x + 2.5) * x - 4) * x + 2, 0.0))


def _resize_matrix(n_in, n_out):
    scale = n_out / n_in
    pos = (np.arange(n_out) + 0.5) / scale - 0.5
    kscale = min(scale, 1.0)
    w = _keys_cubic((np.arange(n_in)[None, :] - pos[:, None]) * kscale)
    return (w / w.sum(axis=1, keepdims=True)).astype(np.float64)


def _band(Dm, t):
    rows = np.nonzero(np.abs(Dm[:, t * 128:(t + 1) * 128]).sum(1) > 0)[0]
    return int(rows.min()), int(rows.max()) + 1


_CACHE = {}


def _build():
    if "nc" in _CACHE:
        return _CACHE["nc"], _CACHE["consts"]

    U = _resize_matrix(H, 2 * H)
    Dm = _resize_matrix(2 * H, H)
    uT = U.T.astype(ml_dtypes.bfloat16)
    dT = np.concatenate([Dm.T[0:128, :], Dm.T[128:256, :]], axis=1)
    dT_bf = dT.astype(ml_dtypes.bfloat16)
    bands = [_band(Dm, 0), _band(Dm, 1)]

    PPI = C // 2                 # pairs per image
    NPAIR = B_CORE * PPI
    PPCH = CPC // 2              # pairs per chunk

    nc = bacc.Bacc()
    x_d = nc.declare_dram_parameter("x", [B_CORE, H, C, W], BF16, isOutput=False)
    ut_d = nc.declare_dram_parameter("ut", [128, 256], BF16, isOutput=False)
    dbf_d = nc.declare_dram_parameter("dbf", [128, 256], BF16, isOutput=False)
    y_d = nc.declare_dram_parameter("y", [B_CORE, H, C, W], BF16, isOutput=True)

    with tile.TileContext(nc) as tc, ExitStack() as ctx:
        wpool = ctx.enter_context(tc.tile_pool(name="weights", bufs=1))
        xpool = ctx.enter_context(tc.tile_pool(name="ximg", bufs=1))
        opool = ctx.enter_context(tc.tile_pool(name="oimg", bufs=2))
        spool = ctx.enter_context(tc.tile_pool(name="sbufs", bufs=3))
        papool = ctx.enter_context(tc.tile_pool(name="pA", bufs=1, space="PSUM"))
        pzpool = ctx.enter_context(tc.tile_pool(name="pZ", bufs=2, space="PSUM"))
        pspool = ctx.enter_context(tc.tile_pool(name="pS", bufs=2, space="PSUM"))
        pypool = ctx.enter_context(tc.tile_pool(name="pY", bufs=1, space="PSUM"))

        ut_s = wpool.tile([128, 256], BF16, tag="ut")
        dbf_s = wpool.tile([128, 256], BF16, tag="dbf")
        nc.sync.dma_start(ut_s[:], ut_d[:])
        nc.sync.dma_start(dbf_s[:], dbf_d[:])

        xch = {}
        och = {}
        live = {}
        cpl = {}     # couple-level tiles: pA2, sP2, pY2

        def dma_in(b, k):
            # 2 tags x pool bufs=1: at most 2 input transfers in flight, so
            # chunk 0 gets ~half the aggregate DMA bandwidth instead of 1/8
            xc = xpool.tile([128, CPC * W], BF16, tag=f"xc{k % 2}",
                            name=f"xc{k % 2}")
            nc.sync.dma_start(
                xc[:],
                x_d[b].rearrange("h c w -> h (c w)")
                [:, k * CPC * W:(k + 1) * CPC * W])
            xch[(b, k)] = xc

        def pair_loc(p):
            b, pi = divmod(p, PPI)
            c0 = pi * 2
            k = c0 // CPC
            cb = (c0 - k * CPC) * W
            return b, pi, k, cb

        def emit_A_mms(p):
            b, pi, k, cb = pair_loc(p)
            pA = papool.tile([128, 512], F32, tag="pA", name="pA")
            for c in range(2):
                nc.tensor.matmul(pA[:, c * 256:(c + 1) * 256],
                                 xch[(b, k)][:, cb + c * W:cb + (c + 1) * W],
                                 ut_s[:], start=True, stop=True)
            sP = spool.tile([128, 512], BF16, tag="sP", name="sP")
            live[p] = {"pA": pA, "sP": sP}

        def emit_A_evac(p):
            nc.vector.tensor_copy(live[p]["sP"][:], live[p]["pA"][:])

        def emit_B_mms(p):
            d = live[p]
            sP = d["sP"][:]
            pZ = pzpool.tile([128, 1024], F32, tag="pZ", name="pZ")
            d["pZ"] = pZ
            return [lambda t=t: nc.tensor.matmul(
                pZ[:, t * 512:(t + 1) * 512],
                ut_s[:, t * 128:(t + 1) * 128],
                sP, start=True, stop=True) for t in range(2)]

        def emit_B_evac(p):
            d = live[p]
            sA = spool.tile([128, 1024], BF16, tag="sA", name="sA")
            nc.scalar.activation(sA[:], d["pZ"][:], AF.Lrelu, alpha=NEG_SLOPE)
            d["sA"] = sA

        def emit_C_mms(p):
            d = live[p]
            sA = d["sA"]
            pS = pspool.tile([128, 512], F32, tag="pS", name="pS")
            d["pS"] = pS
            mms = []
            for c in range(2):
                for m in range(2):
                    for t in range(2):
                        lo, hi = bands[t]
                        mms.append(lambda c=c, m=m, t=t, lo=lo, hi=hi:
                                   nc.tensor.matmul(
                                       pS[:, c * 256 + m * 128 + lo:
                                          c * 256 + m * 128 + hi],
                                       sA[:, t * 512 + c * 256 + m * 128:
                                          t * 512 + c * 256 + (m + 1) * 128],
                                       dbf_s[:, t * 128 + lo:t * 128 + hi],
                                       start=(t == 0), stop=(t == 1),
                                       skip_group_check=True))
            return mms

        def emit_C_evac(p):
            d = live[p]
            sS = spool.tile([128, 512], BF16, tag="sS", name="sS")
            nc.vector.tensor_copy(sS[:], d["pS"][:])
            d["sS"] = sS

        def emit_D_mms(p):
            d = live[p]
            h = p % 2
            if h == 0:
                cpl["pY2"] = pypool.tile([128, 512], F32, tag="pY2",
                                         name="pY2")
            pY2 = cpl["pY2"]
            d["pY2"] = pY2
            sSg = d["sS"][:].rearrange("q (c mw) -> q c mw", c=2)
            return [lambda m=m: nc.tensor.matmul(
                pY2[:, h * 256:(h + 1) * 256],
                dbf_s[:, m * 128:(m + 1) * 128],
                sSg[:, :, m * 128:(m + 1) * 128],
                start=(m == 0), stop=(m == 1),
                skip_group_check=True) for m in range(2)]

        def emit_D_evac(p):
            b, pi, k, cb = pair_loc(p)
            if p % 2 == 0:
                return
            d = live[p]
            if (b, k) not in och:
                och[(b, k)] = opool.tile([128, CPC * W], BF16,
                                         tag=f"oc{k}", name=f"oc{k}")
            # couple dst: cols for pairs p-1, p are contiguous (cb-256..cb+256)
            nc.scalar.copy(och[(b, k)][:, cb - 256:cb + 256], d["pY2"][:])
            if pi % (PPCH // 2) == (PPCH // 2) - 1:   # half-chunk -> DMA out
                base = k * CPC * W + (pi % PPCH // (PPCH // 2)) * (CPC * W // 2)
                nc.sync.dma_start(
                    y_d[b].rearrange("h c w -> h (c w)")
                    [:, base:base + CPC * W // 2],
                    och[(b, k)][:, (base - k * CPC * W):
                                (base - k * CPC * W) + CPC * W // 2])
            del live[p - 1]
            del live[p]

        # input DMA schedule: chunk (b,k) emitted at iteration sched[(b,k)]
        dma_sched = {}
        for k in range(NCHUNK):
            dma_sched.setdefault(0 if k < 2 else 2 * k, []).append((0, k))
        for k in range(NCHUNK):
            dma_sched.setdefault(max(1, PPI - 18 + 2 * k), []).append((1, k))

        SKEW = 3
        for i in range(NPAIR + SKEW):
            for (b, k) in dma_sched.get(i, ()):
                dma_in(b, k)
            # --- PE matmul emission, interleaved for LDW prefetch
            c_mms = emit_C_mms(i - 2) if 0 <= i - 2 < NPAIR else []
            d_mms = emit_D_mms(i - 3) if 0 <= i - 3 < NPAIR else []
            b_mms = emit_B_mms(i - 1) if 0 <= i - 1 < NPAIR else []
            if i < NPAIR:
                emit_A_mms(i)           # 2 MMs (long, cover C LDWs)
                emit_A_evac(i)          # DVE FD-1024 on odd i; must precede
                                        # the B-mms of pair i-1 in emission
            seq = []
            seq += c_mms[0:1] + d_mms[0:1] + c_mms[1:3]
            seq += d_mms[1:2] + c_mms[3:5]
            seq += b_mms[0:1] + c_mms[5:7]
            seq += b_mms[1:2] + c_mms[7:8]
            for mm in seq:
                mm()
            if 0 <= i - 2 < NPAIR:
                emit_C_evac(i - 2)      # DVE FD-512
            if 0 <= i - 1 < NPAIR:
                emit_B_evac(i - 1)      # ACT Lrelu FD-1024
            if 0 <= i - 3 < NPAIR:
                emit_D_evac(i - 3)      # split FD-256 DVE/ACT on odd pairs

    nc.compile()
    consts = {"ut": np.ascontiguousarray(uT),
              "dbf": np.ascontiguousarray(dT_bf)}
    _CACHE["nc"] = nc
    _CACHE["consts"] = consts
    return nc, consts


def kernel(x, in_size=128, out_size=128, trace=False, tmpdir=None):
    x = np.asarray(x, dtype=np.float32)
    assert x.shape == (16, H, W, C), x.shape
    nc, consts = _build()
    xt = np.ascontiguousarray(x.transpose(0, 1, 3, 2))
    in_maps = []
    for core in range(N_CORES):
        m = {"x": np.ascontiguousarray(
            xt[core * B_CORE:(core + 1) * B_CORE]).astype(ml_dtypes.bfloat16)}
        m.update(consts)
        in_maps.append(m)
    res = run_bass_kernel_spmd(nc, in_maps, list(range(N_CORES)), trace=trace,
                               tmpdir=tmpdir)
    out = np.concatenate([res.results[i]["y"] for i in range(N_CORES)], axis=0)
    out = out.astype(np.float32).transpose(0, 1, 3, 2)
    if trace:
        kernel.last_exec_time_ns = res.exec_time_ns
        kernel.last_results = res
    return np.ascontiguousarray(out)


# revision 31
# speedup vs baseline: 1.1937x; 1.1937x over previous
"""v5: skewed pair pipeline + pair-couple merged A/D evacs + PE LDW interleave.

Per channel c: Y = D @ f(U @ X @ U.T) @ D.T, bf16, 2 channels per pair,
pairs software-pipelined with 3-stage skew.  Pair-couples (2j, 2j+1) share
a [128,1024] pA tile (one FD-1024 A-evac) and a [128,512] pY tile (one
FD-512 D-evac).  PE emission interleaves the 8 tiny banded C-MMs between
longer A/B/D MMs so LDWEIGHTS prefetch stays ahead.

Host layout [B,H,C,W]; input chunks' DMAs spread over early iterations so
chunk 0 completes first; output DMA per half-chunk (4 pairs).

PSUM banks: pA2 [128,1024] + pZ [128,1024]x2 + pS [128,512] + pY [128,512] = 8.
Evac: DVE {A-merged FD-1024 (odd pairs), C FD-512}; ACT {Lrelu FD-1024,
D-merged FD-512 (every other)}.
"""
import numpy as np
import ml_dtypes
from contextlib import ExitStack

import concourse.bacc as bacc
import concourse.tile as tile
from concourse import mybir
from concourse.bass_utils import run_bass_kernel_spmd

F32 = mybir.dt.float32
BF16 = mybir.dt.bfloat16
AF = mybir.ActivationFunctionType

N_CORES = 8
B_CORE = 2
H = W = C = 128
NEG_SLOPE = 0.01
NCHUNK = 8
CPC = C // NCHUNK


def _keys_cubic(x):
    x = np.abs(x)
    return np.where(
        x <= 1, (1.5 * x - 2.5) * x * x + 1,
        np.where(x < 2, ((-0.5 * x + 2.5) * x - 4) * x + 2, 0.0))


def _resize_matrix(n_in, n_out):
    scale = n_out / n_in
    pos = (np.arange(n_out) + 0.5) / scale - 0.5
    kscale = min(scale, 1.0)
    w = _keys_cubic((np.arange(n_in)[None, :] - pos[:, None]) * kscale)
    return (w / w.sum(axis=1, keepdims=True)).astype(np.float64)


def _band(Dm, t):
    rows = np.nonzero(np.abs(Dm[:, t * 128:(t + 1) * 128]).sum(1) > 0)[0]
    return int(rows.min()), int(rows.max()) + 1


_CACHE = {}


def _build():
    if "nc" in _CACHE:
        return _CACHE["nc"], _CACHE["consts"]

    U = _resize_matrix(H, 2 * H)
    Dm = _resize_matrix(2 * H, H)
    uT = U.T.astype(ml_dtypes.bfloat16)
    dT = np.concatenate([Dm.T[0:128, :], Dm.T[128:256, :]], axis=1)
    dT_bf = dT.astype(ml_dtypes.bfloat16)
    bands = [_band(Dm, 0), _band(Dm, 1)]

    PPI = C // 2                 # pairs per image
    NPAIR = B_CORE * PPI
    PPCH = CPC // 2              # pairs per chunk

    nc = bacc.Bacc()
    x_d = nc.declare_dram_parameter("x", [B_CORE, H, C, W], BF16, isOutput=False)
    ut_d = nc.declare_dram_parameter("ut", [128, 256], BF16, isOutput=False)
    dbf_d = nc.declare_dram_parameter("dbf", [128, 256], BF16, isOutput=False)
    y_d = nc.declare_dram_parameter("y", [B_CORE, H, C, W], BF16, isOutput=True)

    with tile.TileContext(nc) as tc, ExitStack() as ctx:
        wpool = ctx.enter_context(tc.tile_pool(name="weights", bufs=1))
        xpool = ctx.enter_context(tc.tile_pool(name="ximg", bufs=1))
        opool = ctx.enter_context(tc.tile_pool(name="oimg", bufs=2))
        spool = ctx.enter_context(tc.tile_pool(name="sbufs", bufs=4))
        papool = ctx.enter_context(tc.tile_pool(name="pA", bufs=1, space="PSUM"))
        pzpool = ctx.enter_context(tc.tile_pool(name="pZ", bufs=2, space="PSUM"))
        pspool = ctx.enter_context(tc.tile_pool(name="pS", bufs=2, space="PSUM"))
        pypool = ctx.enter_context(tc.tile_pool(name="pY", bufs=1, space="PSUM"))

        ut_s = wpool.tile([128, 256], BF16, tag="ut")
        dbf_s = wpool.tile([128, 256], BF16, tag="dbf")
        nc.sync.dma_start(ut_s[:], ut_d[:])
        nc.sync.dma_start(dbf_s[:], dbf_d[:])

        xch = {}
        och = {}
        live = {}
        cpl = {}     # couple-level tiles: pA2, sP2, pY2

        def dma_in(b, k):
            # 2 tags x pool bufs=1: at most 2 input transfers in flight, so
            # chunk 0 gets ~half the aggregate DMA bandwidth instead of 1/8
            xc = xpool.tile([128, CPC * W], BF16, tag=f"xc{k % 2}",
                            name=f"xc{k % 2}")
            nc.sync.dma_start(
                xc[:],
                x_d[b].rearrange("h c w -> h (c w)")
                [:, k * CPC * W:(k + 1) * CPC * W])
            xch[(b, k)] = xc

        def pair_loc(p):
            b, pi = divmod(p, PPI)
            c0 = pi * 2
            k = c0 // CPC
            cb = (c0 - k * CPC) * W
            return b, pi, k, cb

        def emit_A_mms(p):
            b, pi, k, cb = pair_loc(p)
            pA = papool.tile([128, 512], F32, tag="pA", name="pA")
            for c in range(2):
                nc.tensor.matmul(pA[:, c * 256:(c + 1) * 256],
                                 xch[(b, k)][:, cb + c * W:cb + (c + 1) * W],
                                 ut_s[:], start=True, stop=True)
            sP = spool.tile([128, 512], BF16, tag="sP", name="sP")
            live[p] = {"pA": pA, "sP": sP}

        def emit_A_evac(p):
            nc.vector.tensor_copy(live[p]["sP"][:], live[p]["pA"][:])

        def emit_B_mms(p):
            d = live[p]
            sP = d["sP"][:]
            pZ = pzpool.tile([128, 1024], F32, tag="pZ", name="pZ")
            d["pZ"] = pZ
            return [lambda t=t: nc.tensor.matmul(
                pZ[:, t * 512:(t + 1) * 512],
                ut_s[:, t * 128:(t + 1) * 128],
                sP, start=True, stop=True) for t in range(2)]

        def emit_B_evac(p):
            d = live[p]
            sA = spool.tile([128, 1024], BF16, tag="sA", name="sA")
            nc.scalar.activation(sA[:], d["pZ"][:], AF.Lrelu, alpha=NEG_SLOPE)
            d["sA"] = sA

        def emit_C_mms(p):
            d = live[p]
            sA = d["sA"]
            pS = pspool.tile([128, 512], F32, tag="pS", name="pS")
            d["pS"] = pS
            mms = []
            for c in range(2):
                for m in range(2):
                    for t in range(2):
                        lo, hi = bands[t]
                        mms.append(lambda c=c, m=m, t=t, lo=lo, hi=hi:
                                   nc.tensor.matmul(
                                       pS[:, c * 256 + m * 128 + lo:
                                          c * 256 + m * 128 + hi],
                                       sA[:, t * 512 + c * 256 + m * 128:
                                          t * 512 + c * 256 + (m + 1) * 128],
                                       dbf_s[:, t * 128 + lo:t * 128 + hi],
                                       start=(t == 0), stop=(t == 1),
                                       skip_group_check=True))
            return mms

        def emit_C_evac(p):
            d = live[p]
            sS = spool.tile([128, 512], BF16, tag="sS", name="sS")
            nc.vector.tensor_copy(sS[:], d["pS"][:])
            d["sS"] = sS

        def emit_D_mms(p):
            d = live[p]
            h = p % 2
            if h == 0:
                cpl["pY2"] = pypool.tile([128, 512], F32, tag="pY2",
                                         name="pY2")
            pY2 = cpl["pY2"]
            d["pY2"] = pY2
            sSg = d["sS"][:].rearrange("q (c mw) -> q c mw", c=2)
            return [lambda m=m: nc.tensor.matmul(
                pY2[:, h * 256:(h + 1) * 256],
                dbf_s[:, m * 128:(m + 1) * 128],
                sSg[:, :, m * 128:(m + 1) * 128],
                start=(m == 0), stop=(m == 1),
                skip_group_check=True) for m in range(2)]

        def emit_D_evac(p):
            b, pi, k, cb = pair_loc(p)
            if p % 2 == 0:
                return
            d = live[p]
            if (b, k) not in och:
                och[(b, k)] = opool.tile([128, CPC * W], BF16,
                                         tag=f"oc{k}", name=f"oc{k}")
            # couple dst: cols for pairs p-1, p are contiguous (cb-256..cb+256)
            nc.scalar.copy(och[(b, k)][:, cb - 256:cb + 256], d["pY2"][:])
            if pi % (PPCH // 2) == (PPCH // 2) - 1:   # half-chunk -> DMA out
                base = k * CPC * W + (pi % PPCH // (PPCH // 2)) * (CPC * W // 2)
                nc.sync.dma_start(
                    y_d[b].rearrange("h c w -> h (c w)")
                    [:, base:base + CPC * W // 2],
                    och[(b, k)][:, (base - k * CPC * W):
                                (base - k * CPC * W) + CPC * W // 2])
            del live[p - 1]
            del live[p]

        # input DMA schedule: chunk (b,k) emitted at iteration sched[(b,k)]
        dma_sched = {}
        for k in range(NCHUNK):
            dma_sched.setdefault(0 if k < 2 else 2 * k, []).append((0, k))
        for k in range(NCHUNK):
            dma_sched.setdefault(max(1, PPI - 18 + 2 * k), []).append((1, k))

        SKEW = 4
        for i in range(NPAIR + SKEW):
            for (b, k) in dma_sched.get(i, ()):
                dma_in(b, k)
            # --- PE matmul emission, interleaved for LDW prefetch
            c_mms = emit_C_mms(i - 3) if 0 <= i - 3 < NPAIR else []
            d_mms = emit_D_mms(i - 4) if 0 <= i - 4 < NPAIR else []
            b_mms = emit_B_mms(i - 2) if 0 <= i - 2 < NPAIR else []
            if i < NPAIR:
                emit_A_mms(i)           # 2 MMs (long, cover C LDWs)
                emit_A_evac(i)          # DVE FD-1024 on odd i; must precede
                                        # the B-mms of pair i-1 in emission
            seq = []
            seq += c_mms[0:1] + d_mms[0:1] + c_mms[1:3]
            seq += d_mms[1:2] + c_mms[3:5]
            seq += b_mms[0:1] + c_mms[5:7]
            seq += b_mms[1:2] + c_mms[7:8]
            for mm in seq:
                mm()
            if 0 <= i - 3 < NPAIR:
                emit_C_evac(i - 3)      # DVE FD-512
            if 0 <= i - 2 < NPAIR:
                emit_B_evac(i - 2)      # ACT Lrelu FD-1024
            if 0 <= i - 4 < NPAIR:
                emit_D_evac(i - 4)      # ACT FD-512 on odd pairs

    nc.compile()
    consts = {"ut": np.ascontiguousarray(uT),
              "dbf": np.ascontiguousarray(dT_bf)}
    _CACHE["nc"] = nc
    _CACHE["consts"] = consts
    return nc, consts


def kernel(x, in_size=128, out_size=128, trace=False, tmpdir=None):
    x = np.asarray(x, dtype=np.float32)
    assert x.shape == (16, H, W, C), x.shape
    nc, consts = _build()
    xt = np.ascontiguousarray(x.transpose(0, 1, 3, 2))
    in_maps = []
    for core in range(N_CORES):
        m = {"x": np.ascontiguousarray(
            xt[core * B_CORE:(core + 1) * B_CORE]).astype(ml_dtypes.bfloat16)}
        m.update(consts)
        in_maps.append(m)
    res = run_bass_kernel_spmd(nc, in_maps, list(range(N_CORES)), trace=trace,
                               tmpdir=tmpdir)
    out = np.concatenate([res.results[i]["y"] for i in range(N_CORES)], axis=0)
    out = out.astype(np.float32).transpose(0, 1, 3, 2)
    if trace:
        kernel.last_exec_time_ns = res.exec_time_ns
        kernel.last_results = res
    return np.ascontiguousarray(out)
